# revision 5
# baseline (speedup 1.0000x reference)
"""GridPoolingLayer kernel for Trainium2 (8 NeuronCores, Bass/Tile).

Semantics: the 1D binary masks partition H/W into maximal runs of constant
value; the layer replaces every grid cell with its mean (keep_size=True).
The op is separable; per core (channels sharded 8 ways, 32 ch/core), fp16.

This version writes only the POOLED grid [S_h, S_w*CS] from the device
(~4.2 MB/core instead of the 16.8 MB expanded output); the broadcast-back
over each cell rectangle is a pure gather done on the host while
unsharding (alongside the column un-permutation the host already does).

Device pipeline, per segment-ALIGNED row chunk (<=128 rows, cut at row
segment starts so no segment straddles chunks; ~5 chunks for H=512):

  matmul   psum[s, (wslot, c)] = sum_h F_k[h, s] * x[h, (wslot, l, c)]
           F_k[h, s] = 1/L_h(s) iff row h is in row-segment s of chunk k.
           The contraction over h does the row pooling; PSUM accumulation
           over the within-col-segment offset l (x is stored l-major per
           column-length class, host permutation) does the col pooling.
           Each input element passes the PE exactly once per pass; one
           full-width pass per chunk.
  evac     y_sb[s, (wslot, c)] = psum * (1/L_w) -- per column-class
           pieces, split between Vector and Scalar engines.
  store    y[s0_k:s1_k, :] <- y_sb, on the Activation HWDGE ring
           (x loads keep the SP ring).

The host un-permutes and broadcasts: out[h, w, c] =
y[rowseg(h), colslot(w)*CS + c], upcast fp16 -> fp32.  fp16 keeps HBM
traffic at 16.8 MB in + 4.2 MB out per core; the 2e-2 harness tolerance
leaves ~40x margin over fp16 rounding noise (col sums accumulate in
fp32 PSUM).
"""

import math
import numpy as np

H, W, C = 512, 512, 256
NCORES = 8
CS = C // NCORES  # 32 channels per core
P = 128
FW = W * CS       # row free size in elements (16384)
PSW = 512         # psum bank width in fp32 elems
CHUNK2 = 1024     # psum tile width (fp32 elems, two banks)
LOAD_SPLIT = 4    # DMAs per row-chunk of x (pipelining granularity)


def _segments(mask):
    m = np.asarray(mask).ravel()
    change = np.nonzero(m[1:] != m[:-1])[0] + 1
    bounds = np.concatenate([[0], change, [len(m)]]).astype(np.int64)
    return [(int(bounds[i]), int(bounds[i + 1])) for i in range(len(bounds) - 1)]


def _plan(row_segs, col_segs):
    """Host-side geometry planning shared by program build + data prep."""
    from collections import defaultdict

    S_h, S_w = len(row_segs), len(col_segs)

    # ---- column side: class-sorted device order -------------------------
    by_len = defaultdict(list)
    for t, (u, v) in enumerate(col_segs):
        by_len[v - u].append(t)

    wperm = np.empty(W, dtype=np.int64)   # dev w unit -> orig w
    perm_cols = []                        # slot -> orig col segment id
    off = 0
    slot_bases = []  # (L, n, slot0) per class, slot-ordered
    sl = 0
    for L in sorted(by_len):
        ts = by_len[L]
        for t in ts:
            u, v = col_segs[t]
            wperm[off:off + L] = np.arange(u, v)
            off += L
            perm_cols.append(t)
        slot_bases.append((L, len(ts), sl))
        sl += len(ts)
    assert off == W and sl == S_w

    # x element layout: class blocks in slot order, each [L, n, CS]
    # l-major; cls_x0[class] = elem offset of the block
    cls_x0 = []
    x0 = 0
    for (L, n, slot0) in slot_bases:
        cls_x0.append(x0)
        x0 += L * n * CS
    assert x0 == FW

    CPW = S_w * CS  # pooled row free size (fp32 psum elems)

    # ---- matmul piece groups, keyed by psum half (512 fp32 = 16 slots) --
    # group: (ps_off, width, [x_off per l]) -- one PSUM-accumulation set
    mm_groups = defaultdict(list)
    for ci_, (L, n, slot0) in enumerate(slot_bases):
        i = 0
        while i < n:
            hi = (slot0 + i) * CS // PSW
            room = ((hi + 1) * PSW - (slot0 + i) * CS) // CS
            take = min(room, n - i)
            x_offs = [
                cls_x0[ci_] + l * n * CS + i * CS for l in range(L)
            ]
            mm_groups[hi].append(
                ((slot0 + i) * CS, take * CS, x_offs)
            )
            i += take

    # ---- evacuation pieces, keyed by psum chunk (CHUNK2) ----------------
    slots_per_chunk = CHUNK2 // CS
    exp_pieces = defaultdict(list)
    for (L, n, slot0) in slot_bases:
        i = 0
        while i < n:
            ch = (slot0 + i) // slots_per_chunk
            room = (ch + 1) * slots_per_chunk - (slot0 + i)
            take = min(room, n - i)
            exp_pieces[ch].append((L, take, slot0 + i))
            i += take

    n_chunks = math.ceil(CPW / CHUNK2)

    # ---- row side: segment-aligned chunks of <=128 rows -----------------
    seg_of_h = np.empty(H, dtype=np.int64)
    for s, (va, vb) in enumerate(row_segs):
        seg_of_h[va:vb] = s
    starts = [a for a, b in row_segs]
    cuts = [0]
    while cuts[-1] < H:
        nxt = max(s for s in starts + [H] if 0 < s - 0 and s <= cuts[-1] + P and s > cuts[-1])
        cuts.append(nxt)
    chunks = []  # (r0, r1, s0, s1)
    for i in range(len(cuts) - 1):
        r0, r1 = cuts[i], cuts[i + 1]
        s0 = int(seg_of_h[r0])
        s1 = int(seg_of_h[r1 - 1]) + 1
        assert r1 - r0 <= P and row_segs[s0][0] == r0 and row_segs[s1 - 1][1] == r1
        chunks.append((r0, r1, s0, s1))
    # process the smallest chunk first: its x load is tiny, so its
    # full-width PE pass overlaps the first big chunk's load
    order = sorted(range(len(chunks)), key=lambda k: chunks[k][1] - chunks[k][0])

    return dict(
        S_h=S_h, S_w=S_w, CPW=CPW,
        wperm=wperm, perm_cols=perm_cols, slot_bases=slot_bases,
        cls_x0=cls_x0,
        mm_groups=mm_groups, exp_pieces=exp_pieces, n_chunks=n_chunks,
        seg_of_h=seg_of_h, chunks=chunks, order=order,
    )


def _build_program(row_segs, col_segs, plan):
    import concourse.mybir as mybir
    import concourse.tile as tile
    from concourse import bacc

    fp16 = mybir.dt.float16
    fp32 = mybir.dt.float32
    COPY = mybir.ActivationFunctionType.Copy

    S_h = plan["S_h"]
    CPW = plan["CPW"]
    chunks = plan["chunks"]
    Kc = len(chunks)
    n_chunks = plan["n_chunks"]
    n_halves = math.ceil(CPW / PSW)

    nc = bacc.Bacc()
    x = nc.dram_tensor("x", [H, FW], fp16, kind="ExternalInput")
    fM = nc.dram_tensor("fM", [Kc * P, P], fp16, kind="ExternalInput")
    y = nc.dram_tensor("y", [S_h, CPW], fp16, kind="ExternalOutput")

    with tile.TileContext(nc) as tc:
        with (
            tc.tile_pool(name="consts", bufs=1) as consts,
            tc.tile_pool(name="ys", bufs=2) as ypool,
            tc.tile_pool(name="ps2", bufs=3, space="PSUM") as ps2pool,
            tc.tile_pool(name="warm", bufs=1, space="PSUM") as warmpool,
        ):
            # per-chunk pooling matrices (stationary), on the SP ring
            fM_sb = []
            for k in range(Kc):
                t = consts.tile([P, P], fp16, name=f"fM{k}")
                nc.sync.dma_start(t[:], fM[k * P:(k + 1) * P, :])
                fM_sb.append(t)

            # x resident in SBUF, one tile per row chunk (smallest chunk
            # first: its tiny load lets the PE start while the first big
            # chunk streams in), loaded in slices on the Activation HWDGE
            # ring (stores keep the SP ring)
            x_sb = {}
            for k in plan["order"]:
                r0, r1, s0, s1 = chunks[k]
                t = consts.tile([P, FW], fp16, name=f"x{k}")
                step = FW // LOAD_SPLIT
                for s0_ in range(0, FW, step):
                    nc.scalar.dma_start(
                        t[:r1 - r0, s0_:s0_ + step],
                        x[r0:r1, s0_:s0_ + step],
                    )
                x_sb[k] = t

            # PE pre-touch of every stationary tile: later matmuls then
            # reach the operand without a DMA wait (keeps the LDWEIGHTS
            # sync-wait count within the ISA limit).
            ps_warm = warmpool.tile([1, PSW], fp32, name="ps_warm")
            for t in fM_sb:
                nc.tensor.matmul(
                    ps_warm[:1, :1], t[:, :1], t[:, :1],
                    start=True, stop=True,
                )

            # --------- pooled pool per row chunk -------------------------
            exp_rr = 0
            for k in plan["order"]:
                r0, r1, s0, s1 = chunks[k]
                rows = r1 - r0
                n_k = s1 - s0
                ot = ypool.tile([P, CPW], fp16, tag="ot", name=f"ot{k}")
                for ci in range(n_chunks):
                    c0 = ci * CHUNK2
                    cw = min(CHUNK2, CPW - c0)
                    ps2 = ps2pool.tile([P, CHUNK2], fp32, tag="ps2",
                                       name=f"ps2_{k}_{ci}")
                    for hi in range(2 * ci, min(2 * ci + 2, n_halves)):
                        for (ps_off, width, x_offs) in plan["mm_groups"][hi]:
                            tot = len(x_offs)
                            for idx, x_off in enumerate(x_offs):
                                nc.tensor.matmul(
                                    ps2[:n_k,
                                        ps_off - c0:
                                        ps_off - c0 + width],
                                    fM_sb[k][:rows, :n_k],
                                    x_sb[k][:rows, x_off:x_off + width],
                                    start=(idx == 0),
                                    stop=(idx == tot - 1),
                                )
                    for (L, n, slot0) in plan["exp_pieces"][ci]:
                        src = ps2[:n_k, slot0 * CS - c0:
                                  (slot0 + n) * CS - c0]
                        dst = ot[:n_k, slot0 * CS:(slot0 + n) * CS]
                        if exp_rr % 2 == 0:
                            nc.vector.tensor_scalar_mul(dst, src, 1.0 / L)
                        else:
                            nc.scalar.activation(dst, src, COPY,
                                                 scale=1.0 / L)
                        exp_rr += 1
                half = (CPW // 2) & ~1
                nc.sync.dma_start(y[s0:s1, :half], ot[:n_k, :half])
                nc.sync.dma_start(y[s0:s1, half:], ot[:n_k, half:])

    nc.compile()
    nc.finalize()
    return nc


def _prep_host(input, h_mask, v_mask):
    """Returns (nc, in_maps, plan) ready for execution."""
    row_segs = _segments(h_mask)
    col_segs = _segments(v_mask)
    plan = _plan(row_segs, col_segs)
    nc = _build_program(row_segs, col_segs, plan)
    in_maps = _make_in_maps(input, row_segs, plan)
    return nc, in_maps, plan


def _make_in_maps(input, row_segs, plan):
    # per-chunk pooling matrices: fM[k*128 + (h - r0), s - s0] = 1/len(seg s)
    chunks = plan["chunks"]
    Kc = len(chunks)
    seg_of_h = plan["seg_of_h"]
    fM = np.zeros((Kc * P, P), dtype=np.float16)
    for k, (r0, r1, s0, s1) in enumerate(chunks):
        for h in range(r0, r1):
            s = int(seg_of_h[h])
            va, vb = row_segs[s]
            fM[k * P + (h - r0), s - s0] = np.float16(1.0 / (vb - va))

    # device x layout: class blocks in slot order, each [L, n, C] l-major
    xp16 = np.asarray(input)[0].astype(np.float16)  # [H, W, C]
    parts = []
    pos = 0
    for (L, n, slot0) in plan["slot_bases"]:
        cols = plan["wperm"][pos:pos + n * L]
        pos += n * L
        blk = xp16[:, cols, :].reshape(H, n, L, C)
        parts.append(np.ascontiguousarray(blk.transpose(0, 2, 1, 3)))

    in_maps = []
    for k in range(NCORES):
        xc = np.concatenate(
            [p[:, :, :, k * CS:(k + 1) * CS].reshape(H, -1) for p in parts],
            axis=1,
        )
        in_maps.append({"x": np.ascontiguousarray(xc), "fM": fM})
    return in_maps


# stash for test.py introspection
LAST_RESULT = {}
_EXEC_CACHE = {}


def _make_executable(nc):
    """Build a reusable sharded jit callable for this program.

    Mirrors bass2jax.run_bass_via_pjrt's multi-core branch but keeps the
    jitted function so repeated calls skip retrace/recompile (and so the
    test harness can time steady-state executions).
    """
    import jax
    import concourse.mybir as mybir
    from concourse import bass2jax
    from jax.sharding import Mesh, PartitionSpec
    from jax.experimental.shard_map import shard_map

    bass2jax.install_neuronx_cc_hook()

    partition_name = (
        nc.partition_id_tensor.name if nc.partition_id_tensor else None
    )
    in_names, out_names, out_shapes, out_dtypes = [], [], [], []
    for alloc in nc.m.functions[0].allocations:
        if not isinstance(alloc, mybir.MemoryLocationSet):
            continue
        name = alloc.memorylocations[0].name
        if alloc.kind == "ExternalInput":
            if name != partition_name:
                in_names.append(name)
        elif alloc.kind == "ExternalOutput":
            out_names.append(name)
            out_shapes.append(tuple(alloc.tensor_shape))
            out_dtypes.append(mybir.dt.np(alloc.dtype))
    out_avals = tuple(
        jax.core.ShapedArray(s, d) for s, d in zip(out_shapes, out_dtypes)
    )
    n_params = len(in_names)
    n_outs = len(out_names)
    all_names = in_names + out_names
    if partition_name is not None:
        all_names = all_names + [partition_name]

    def _body(*args):
        operands = list(args)
        if partition_name is not None:
            operands.append(bass2jax.partition_id_tensor())
        outs = bass2jax._bass_exec_p.bind(
            *operands,
            out_avals=out_avals,
            in_names=tuple(all_names),
            out_names=tuple(out_names),
            lowering_input_output_aliases=(),
            sim_require_finite=True,
            sim_require_nnan=True,
            nc=nc,
        )
        return tuple(outs)

    devices = jax.devices()[:NCORES]
    mesh = Mesh(np.asarray(devices), ("core",))
    donate = tuple(range(n_params, n_params + n_outs))
    sharded = jax.jit(
        shard_map(
            _body,
            mesh=mesh,
            in_specs=(PartitionSpec("core"),) * (n_params + n_outs),
            out_specs=(PartitionSpec("core"),) * n_outs,
            check_rep=False,
        ),
        donate_argnums=donate,
        keep_unused=True,
    )

    def run(in_maps):
        concat_in = [
            np.concatenate([m[name] for m in in_maps], axis=0)
            for name in in_names
        ]
        concat_zeros = [
            np.zeros((NCORES * s[0], *s[1:]), d)
            for s, d in zip(out_shapes, out_dtypes)
        ]
        out_arrs = sharded(*concat_in, *concat_zeros)
        return [
            {
                name: np.asarray(out_arrs[i]).reshape(
                    NCORES, *out_shapes[i]
                )[c]
                for i, name in enumerate(out_names)
            }
            for c in range(NCORES)
        ]

    return run


def _get_run(input, h_mask, v_mask):
    key = (np.asarray(h_mask).tobytes(), np.asarray(v_mask).tobytes())
    if key not in _EXEC_CACHE:
        nc, in_maps, plan = _prep_host(
            np.asarray(input), np.asarray(h_mask), np.asarray(v_mask)
        )
        LAST_RESULT["nc"] = nc
        _EXEC_CACHE[key] = (_make_executable(nc), plan)
    else:
        row_segs = _segments(h_mask)
        plan = _EXEC_CACHE[key][1]
        in_maps = _make_in_maps(np.asarray(input), row_segs, plan)
    return _EXEC_CACHE[key][0], in_maps


def kernel(input, h_mask, v_mask):
    run, in_maps = _get_run(input, h_mask, v_mask)
    results = run(in_maps)
    LAST_RESULT["results"] = results

    key = (np.asarray(h_mask).tobytes(), np.asarray(v_mask).tobytes())
    plan = _EXEC_CACHE[key][1]
    S_h, S_w = plan["S_h"], plan["S_w"]

    # broadcast-back gather: out[h, w, c] = y[rowseg(h), colslot(w), c]
    slot_of_seg = np.empty(S_w, dtype=np.int64)
    slot_of_seg[np.asarray(plan["perm_cols"], dtype=np.int64)] = \
        np.arange(S_w)
    col_segs = _segments(v_mask)
    seg_of_w = np.empty(W, dtype=np.int64)
    for t, (u, v) in enumerate(col_segs):
        seg_of_w[u:v] = t
    col_ix = slot_of_seg[seg_of_w]          # [W] -> slot
    row_ix = plan["seg_of_h"]               # [H] -> row segment

    out = np.empty((H, W, C), dtype=np.float32)
    for k in range(NCORES):
        yk = results[k]["y"].astype(np.float32).reshape(S_h, S_w, CS)
        out[:, :, k * CS:(k + 1) * CS] = yk[row_ix][:, col_ix]
    return out[None]


# revision 9
# speedup vs baseline: 1.2370x; 1.2370x over previous
"""GridPoolingLayer kernel for Trainium2 (8 NeuronCores, Bass/Tile).

Semantics: the 1D binary masks partition H/W into maximal runs of constant
value; the layer replaces every grid cell with its mean (keep_size=True).
The op is separable; per core (channels sharded 8 ways, 32 ch/core), fp16.

This version writes only the POOLED grid [S_h, S_w*CS] from the device
(~4.2 MB/core instead of the 16.8 MB expanded output); the broadcast-back
over each cell rectangle is a pure gather done on the host while
unsharding (alongside the column un-permutation the host already does).

Device pipeline, per segment-ALIGNED row chunk (<=128 rows, cut at row
segment starts so no segment straddles chunks; ~5 chunks for H=512):

  matmul   psum[s, (wslot, c)] = sum_h F_k[h, s] * x[h, (wslot, l, c)]
           F_k[h, s] = 1/L_h(s) iff row h is in row-segment s of chunk k.
           The contraction over h does the row pooling; PSUM accumulation
           over the within-col-segment offset l (x is stored l-major per
           column-length class, host permutation) does the col pooling.
           Each input element passes the PE exactly once per pass; one
           full-width pass per chunk.
  evac     y_sb[s, (wslot, c)] = psum * (1/L_w) -- per column-class
           pieces, split between Vector and Scalar engines.
  store    y[s0_k:s1_k, :] <- y_sb, on the Activation HWDGE ring
           (x loads keep the SP ring).

The host un-permutes and broadcasts: out[h, w, c] =
y[rowseg(h), colslot(w)*CS + c], upcast fp16 -> fp32.  fp16 keeps HBM
traffic at 16.8 MB in + 4.2 MB out per core; the 2e-2 harness tolerance
leaves ~40x margin over fp16 rounding noise (col sums accumulate in
fp32 PSUM).
"""

import math
import numpy as np

H, W, C = 512, 512, 256
NCORES = 8
CS = C // NCORES  # 32 channels per core
P = 128
FW = W * CS       # row free size in elements (16384)
PSW = 512         # psum bank width in fp32 elems
CHUNK2 = 1024     # psum tile width (fp32 elems, two banks)
LOAD_SPLIT = 4    # DMAs per row-chunk of x (pipelining granularity)


def _segments(mask):
    m = np.asarray(mask).ravel()
    change = np.nonzero(m[1:] != m[:-1])[0] + 1
    bounds = np.concatenate([[0], change, [len(m)]]).astype(np.int64)
    return [(int(bounds[i]), int(bounds[i + 1])) for i in range(len(bounds) - 1)]


def _plan(row_segs, col_segs):
    """Host-side geometry planning shared by program build + data prep."""
    from collections import defaultdict

    S_h, S_w = len(row_segs), len(col_segs)

    # ---- column side: class-sorted device order -------------------------
    by_len = defaultdict(list)
    for t, (u, v) in enumerate(col_segs):
        by_len[v - u].append(t)

    wperm = np.empty(W, dtype=np.int64)   # dev w unit -> orig w
    perm_cols = []                        # slot -> orig col segment id
    off = 0
    slot_bases = []  # (L, n, slot0) per class, slot-ordered
    sl = 0
    for L in sorted(by_len):
        ts = by_len[L]
        for t in ts:
            u, v = col_segs[t]
            wperm[off:off + L] = np.arange(u, v)
            off += L
            perm_cols.append(t)
        slot_bases.append((L, len(ts), sl))
        sl += len(ts)
    assert off == W and sl == S_w

    # x element layout: class blocks in slot order, each [L, n, CS]
    # l-major; cls_x0[class] = elem offset of the block
    cls_x0 = []
    x0 = 0
    for (L, n, slot0) in slot_bases:
        cls_x0.append(x0)
        x0 += L * n * CS
    assert x0 == FW

    CPW = S_w * CS  # pooled row free size (fp32 psum elems)

    # ---- matmul piece groups, keyed by psum half (512 fp32 = 16 slots) --
    # group: (ps_off, width, [x_off per l]) -- one PSUM-accumulation set
    mm_groups = defaultdict(list)
    for ci_, (L, n, slot0) in enumerate(slot_bases):
        i = 0
        while i < n:
            hi = (slot0 + i) * CS // PSW
            room = ((hi + 1) * PSW - (slot0 + i) * CS) // CS
            take = min(room, n - i)
            x_offs = [
                cls_x0[ci_] + l * n * CS + i * CS for l in range(L)
            ]
            mm_groups[hi].append(
                ((slot0 + i) * CS, take * CS, x_offs)
            )
            i += take

    # ---- evacuation pieces, keyed by psum chunk (CHUNK2) ----------------
    slots_per_chunk = CHUNK2 // CS
    exp_pieces = defaultdict(list)
    for (L, n, slot0) in slot_bases:
        i = 0
        while i < n:
            ch = (slot0 + i) // slots_per_chunk
            room = (ch + 1) * slots_per_chunk - (slot0 + i)
            take = min(room, n - i)
            exp_pieces[ch].append((L, take, slot0 + i))
            i += take

    n_chunks = math.ceil(CPW / CHUNK2)

    # ---- row side: pack whole segments into bins of <=128 rows ----------
    # Each bin is one PE pass (one full-width stream of x), so fewer bins
    # directly cut PE time.  ceil(H/128) = 4 bins requires a perfect
    # packing; contiguous cuts rarely give one, so adopt the leftover
    # tail segments into bins with spare rows (extra row-runs only cost
    # an extra DMA slice each).
    seg_of_h = np.empty(H, dtype=np.int64)
    for s, (va, vb) in enumerate(row_segs):
        seg_of_h[va:vb] = s
    bins = _pack_bins(row_segs, seg_of_h)

    return dict(
        S_h=S_h, S_w=S_w, CPW=CPW,
        wperm=wperm, perm_cols=perm_cols, slot_bases=slot_bases,
        cls_x0=cls_x0,
        mm_groups=mm_groups, exp_pieces=exp_pieces, n_chunks=n_chunks,
        seg_of_h=seg_of_h, bins=bins,
    )


def _pack_bins(row_segs, seg_of_h):
    """Pack whole row segments into ceil(H/128) bins of <=128 rows.

    Returns a list of bins; each bin is a list of segment ids (ordered so
    that row-contiguous segments stay adjacent).  Falls back to greedy
    contiguous chunks (one extra bin) if no 4-bin packing is found.
    """
    S = len(row_segs)
    sizes = [b - a for a, b in row_segs]
    starts = [a for a, b in row_segs]
    n_bins = math.ceil(H / P)

    def greedy(cap):
        cuts = [0]
        while cuts[-1] < H:
            cands = [s for s in starts + [H] if cuts[-1] < s <= cuts[-1] + cap]
            if not cands:
                return None
            cuts.append(max(cands))
        return cuts

    sol = None
    # DFS over contiguous cut choices for the first n_bins bins; leftover
    # tail segments are first-fit-decreasing'd into bins' spare rows.
    cand_lim = 6

    def dfs(cuts):
        nonlocal sol
        if sol is not None:
            return
        if len(cuts) == n_bins + 1:
            c_end = cuts[-1]
            spare = [P - (cuts[i + 1] - cuts[i]) for i in range(n_bins)]
            tail = [s for s in range(S) if row_segs[s][0] >= c_end]
            assign = [[] for _ in range(n_bins)]
            for s in sorted(tail, key=lambda s: -sizes[s]):
                for b in range(n_bins):
                    if spare[b] >= sizes[s]:
                        spare[b] -= sizes[s]
                        assign[b].append(s)
                        break
                else:
                    return
            bins = []
            for b in range(n_bins):
                segs = [s for s in range(S)
                        if cuts[b] <= row_segs[s][0] < cuts[b + 1]]
                bins.append(segs + sorted(assign[b]))
            sol = bins
            return
        last = cuts[-1]
        cands = sorted(
            [s for s in starts if last < s <= last + P], reverse=True
        )[:cand_lim]
        for c in cands:
            dfs(cuts + [c])
            if sol is not None:
                return

    dfs([0])
    if sol is not None:
        return sol
    # fallback: greedy contiguous bins (usually n_bins + 1 of them),
    # smallest first so the tiny pass overlaps the first big load
    cuts = greedy(P)
    bins = []
    for i in range(len(cuts) - 1):
        bins.append([s for s in range(S)
                     if cuts[i] <= row_segs[s][0] < cuts[i + 1]])
    bins.sort(key=lambda segs: sum(sizes[s] for s in segs))
    return bins


def _bin_geometry(row_segs, bins):
    """Per-bin packing info: row runs, F matrix row/col maps.

    Returns list of dicts with:
      runs:  [(r0, r1, p0)] row ranges and their partition base
      rows:  total packed rows
      segs:  segment ids in bin order (psum slot / output row order)
    """
    out = []
    for segs in bins:
        runs = []
        p = 0
        cur = None
        for s in segs:
            a, b = row_segs[s]
            if cur is not None and a == cur[1]:
                cur = (cur[0], b, cur[2])
            else:
                if cur is not None:
                    runs.append(cur)
                    p += cur[1] - cur[0]
                cur = (a, b, p)
        if cur is not None:
            runs.append(cur)
            p += cur[1] - cur[0]
        assert p <= P
        out.append(dict(runs=runs, rows=p, segs=list(segs)))
    return out


def _build_program(row_segs, col_segs, plan):
    import concourse.mybir as mybir
    import concourse.tile as tile
    from concourse import bacc

    fp16 = mybir.dt.float16
    fp32 = mybir.dt.float32
    COPY = mybir.ActivationFunctionType.Copy

    S_h = plan["S_h"]
    CPW = plan["CPW"]
    geos = _bin_geometry(row_segs, plan["bins"])
    Kc = len(geos)
    n_chunks = plan["n_chunks"]
    n_halves = math.ceil(CPW / PSW)
    # pipelined stores: one DMA per group of psum chunks, issued as soon
    # as that group's evacuations are done (more descriptors in flight =
    # more DMA-engine parallelism; short tail after the last pass)
    STORE_SPLIT = 4
    store_bounds = [
        min(CPW, ((i * n_chunks + STORE_SPLIT - 1) // STORE_SPLIT) * CHUNK2)
        for i in range(STORE_SPLIT + 1)
    ]

    nc = bacc.Bacc()
    x = nc.dram_tensor("x", [H, FW], fp16, kind="ExternalInput")
    fM = nc.dram_tensor("fM", [Kc * P, P], fp16, kind="ExternalInput")
    y = nc.dram_tensor("y", [S_h, CPW], fp16, kind="ExternalOutput")

    with tile.TileContext(nc) as tc:
        with (
            tc.tile_pool(name="consts", bufs=1) as consts,
            tc.tile_pool(name="ys", bufs=2) as ypool,
            tc.tile_pool(name="ps2", bufs=3, space="PSUM") as ps2pool,
            tc.tile_pool(name="warm", bufs=1, space="PSUM") as warmpool,
        ):
            # per-bin pooling matrices (stationary), on the SP ring
            fM_sb = []
            for k in range(Kc):
                t = consts.tile([P, P], fp16, name=f"fM{k}")
                nc.sync.dma_start(t[:], fM[k * P:(k + 1) * P, :])
                fM_sb.append(t)

            # x resident in SBUF, one tile per bin, loaded run-by-run in
            # column slices on the Activation HWDGE ring (stores keep the
            # SP ring)
            x_sb = []
            for k, g in enumerate(geos):
                t = consts.tile([P, FW], fp16, name=f"x{k}")
                step = FW // LOAD_SPLIT
                for s0_ in range(0, FW, step):
                    for (r0, r1, p0) in g["runs"]:
                        nc.scalar.dma_start(
                            t[p0:p0 + r1 - r0, s0_:s0_ + step],
                            x[r0:r1, s0_:s0_ + step],
                        )
                x_sb.append(t)

            # PE pre-touch of every stationary tile: later matmuls then
            # reach the operand without a DMA wait (keeps the LDWEIGHTS
            # sync-wait count within the ISA limit).
            ps_warm = warmpool.tile([1, PSW], fp32, name="ps_warm")
            for t in fM_sb:
                nc.tensor.matmul(
                    ps_warm[:1, :1], t[:, :1], t[:, :1],
                    start=True, stop=True,
                )

            # --------- pooled pass per bin -------------------------------
            exp_rr = 0
            N0 = 0
            for k, g in enumerate(geos):
                rows = g["rows"]
                n_k = len(g["segs"])
                ot = ypool.tile([P, CPW], fp16, tag="ot", name=f"ot{k}")
                si = 0
                for ci in range(n_chunks):
                    c0 = ci * CHUNK2
                    ps2 = ps2pool.tile([P, CHUNK2], fp32, tag="ps2",
                                       name=f"ps2_{k}_{ci}")
                    for hi in range(2 * ci, min(2 * ci + 2, n_halves)):
                        for (ps_off, width, x_offs) in plan["mm_groups"][hi]:
                            tot = len(x_offs)
                            for idx, x_off in enumerate(x_offs):
                                nc.tensor.matmul(
                                    ps2[:n_k,
                                        ps_off - c0:
                                        ps_off - c0 + width],
                                    fM_sb[k][:rows, :n_k],
                                    x_sb[k][:rows, x_off:x_off + width],
                                    start=(idx == 0),
                                    stop=(idx == tot - 1),
                                )
                    for (L, n, slot0) in plan["exp_pieces"][ci]:
                        src = ps2[:n_k, slot0 * CS - c0:
                                  (slot0 + n) * CS - c0]
                        dst = ot[:n_k, slot0 * CS:(slot0 + n) * CS]
                        if exp_rr % 2 == 0:
                            nc.vector.tensor_scalar_mul(dst, src, 1.0 / L)
                        else:
                            nc.scalar.activation(dst, src, COPY,
                                                 scale=1.0 / L)
                        exp_rr += 1
                    while (si < STORE_SPLIT
                           and (ci + 1) * CHUNK2 >= store_bounds[si + 1]):
                        w0, w1 = store_bounds[si], store_bounds[si + 1]
                        nc.sync.dma_start(y[N0:N0 + n_k, w0:w1],
                                          ot[:n_k, w0:w1])
                        si += 1
                N0 += n_k

    nc.compile()
    nc.finalize()
    return nc


def _prep_host(input, h_mask, v_mask):
    """Returns (nc, in_maps, plan) ready for execution."""
    row_segs = _segments(h_mask)
    col_segs = _segments(v_mask)
    plan = _plan(row_segs, col_segs)
    nc = _build_program(row_segs, col_segs, plan)
    in_maps = _make_in_maps(input, row_segs, plan)
    return nc, in_maps, plan


def _make_in_maps(input, row_segs, plan):
    # per-bin pooling matrices: fM[k*128 + packed_row, local_slot] = 1/len
    geos = _bin_geometry(row_segs, plan["bins"])
    Kc = len(geos)
    fM = np.zeros((Kc * P, P), dtype=np.float16)
    for k, g in enumerate(geos):
        slot = {s: i for i, s in enumerate(g["segs"])}
        for (r0, r1, p0) in g["runs"]:
            for h in range(r0, r1):
                s = int(plan["seg_of_h"][h])
                va, vb = row_segs[s]
                fM[k * P + p0 + (h - r0), slot[s]] = np.float16(
                    1.0 / (vb - va))

    # device x layout: class blocks in slot order, each [L, n, C] l-major
    xp16 = np.asarray(input)[0].astype(np.float16)  # [H, W, C]
    parts = []
    pos = 0
    for (L, n, slot0) in plan["slot_bases"]:
        cols = plan["wperm"][pos:pos + n * L]
        pos += n * L
        blk = xp16[:, cols, :].reshape(H, n, L, C)
        parts.append(np.ascontiguousarray(blk.transpose(0, 2, 1, 3)))

    in_maps = []
    for k in range(NCORES):
        xc = np.concatenate(
            [p[:, :, :, k * CS:(k + 1) * CS].reshape(H, -1) for p in parts],
            axis=1,
        )
        in_maps.append({"x": np.ascontiguousarray(xc), "fM": fM})
    return in_maps


# stash for test.py introspection
LAST_RESULT = {}
_EXEC_CACHE = {}


def _make_executable(nc):
    """Build a reusable sharded jit callable for this program.

    Mirrors bass2jax.run_bass_via_pjrt's multi-core branch but keeps the
    jitted function so repeated calls skip retrace/recompile (and so the
    test harness can time steady-state executions).
    """
    import jax
    import concourse.mybir as mybir
    from concourse import bass2jax
    from jax.sharding import Mesh, PartitionSpec
    from jax.experimental.shard_map import shard_map

    bass2jax.install_neuronx_cc_hook()

    partition_name = (
        nc.partition_id_tensor.name if nc.partition_id_tensor else None
    )
    in_names, out_names, out_shapes, out_dtypes = [], [], [], []
    for alloc in nc.m.functions[0].allocations:
        if not isinstance(alloc, mybir.MemoryLocationSet):
            continue
        name = alloc.memorylocations[0].name
        if alloc.kind == "ExternalInput":
            if name != partition_name:
                in_names.append(name)
        elif alloc.kind == "ExternalOutput":
            out_names.append(name)
            out_shapes.append(tuple(alloc.tensor_shape))
            out_dtypes.append(mybir.dt.np(alloc.dtype))
    out_avals = tuple(
        jax.core.ShapedArray(s, d) for s, d in zip(out_shapes, out_dtypes)
    )
    n_params = len(in_names)
    n_outs = len(out_names)
    all_names = in_names + out_names
    if partition_name is not None:
        all_names = all_names + [partition_name]

    def _body(*args):
        operands = list(args)
        if partition_name is not None:
            operands.append(bass2jax.partition_id_tensor())
        outs = bass2jax._bass_exec_p.bind(
            *operands,
            out_avals=out_avals,
            in_names=tuple(all_names),
            out_names=tuple(out_names),
            lowering_input_output_aliases=(),
            sim_require_finite=True,
            sim_require_nnan=True,
            nc=nc,
        )
        return tuple(outs)

    devices = jax.devices()[:NCORES]
    mesh = Mesh(np.asarray(devices), ("core",))
    donate = tuple(range(n_params, n_params + n_outs))
    sharded = jax.jit(
        shard_map(
            _body,
            mesh=mesh,
            in_specs=(PartitionSpec("core"),) * (n_params + n_outs),
            out_specs=(PartitionSpec("core"),) * n_outs,
            check_rep=False,
        ),
        donate_argnums=donate,
        keep_unused=True,
    )

    def run(in_maps):
        concat_in = [
            np.concatenate([m[name] for m in in_maps], axis=0)
            for name in in_names
        ]
        concat_zeros = [
            np.zeros((NCORES * s[0], *s[1:]), d)
            for s, d in zip(out_shapes, out_dtypes)
        ]
        out_arrs = sharded(*concat_in, *concat_zeros)
        return [
            {
                name: np.asarray(out_arrs[i]).reshape(
                    NCORES, *out_shapes[i]
                )[c]
                for i, name in enumerate(out_names)
            }
            for c in range(NCORES)
        ]

    return run


def _get_run(input, h_mask, v_mask):
    key = (np.asarray(h_mask).tobytes(), np.asarray(v_mask).tobytes())
    if key not in _EXEC_CACHE:
        nc, in_maps, plan = _prep_host(
            np.asarray(input), np.asarray(h_mask), np.asarray(v_mask)
        )
        LAST_RESULT["nc"] = nc
        _EXEC_CACHE[key] = (_make_executable(nc), plan)
    else:
        row_segs = _segments(h_mask)
        plan = _EXEC_CACHE[key][1]
        in_maps = _make_in_maps(np.asarray(input), row_segs, plan)
    return _EXEC_CACHE[key][0], in_maps


def kernel(input, h_mask, v_mask):
    run, in_maps = _get_run(input, h_mask, v_mask)
    results = run(in_maps)
    LAST_RESULT["results"] = results

    key = (np.asarray(h_mask).tobytes(), np.asarray(v_mask).tobytes())
    plan = _EXEC_CACHE[key][1]
    S_h, S_w = plan["S_h"], plan["S_w"]

    # broadcast-back gather: out[h, w, c] = y[devrow(rowseg(h)), colslot(w), c]
    slot_of_seg = np.empty(S_w, dtype=np.int64)
    slot_of_seg[np.asarray(plan["perm_cols"], dtype=np.int64)] = \
        np.arange(S_w)
    col_segs = _segments(v_mask)
    seg_of_w = np.empty(W, dtype=np.int64)
    for t, (u, v) in enumerate(col_segs):
        seg_of_w[u:v] = t
    col_ix = slot_of_seg[seg_of_w]          # [W] -> slot
    devrow_of_seg = np.empty(S_h, dtype=np.int64)
    n = 0
    for segs in plan["bins"]:
        for s in segs:
            devrow_of_seg[s] = n
            n += 1
    assert n == S_h
    row_ix = devrow_of_seg[plan["seg_of_h"]]  # [H] -> device y row

    out = np.empty((H, W, C), dtype=np.float32)
    for k in range(NCORES):
        yk = results[k]["y"].astype(np.float32).reshape(S_h, S_w, CS)
        out[:, :, k * CS:(k + 1) * CS] = yk[row_ix][:, col_ix]
    return out[None]


# revision 14
# speedup vs baseline: 1.2866x; 1.0401x over previous
"""GridPoolingLayer kernel for Trainium2 (8 NeuronCores, Bass/Tile).

Semantics: the 1D binary masks partition H/W into maximal runs of constant
value; the layer replaces every grid cell with its mean (keep_size=True).
The op is separable; per core (channels sharded 8 ways, 32 ch/core), fp16.

This version writes only the POOLED grid [S_h, S_w*CS] from the device
(~4.2 MB/core instead of the 16.8 MB expanded output); the broadcast-back
over each cell rectangle is a pure gather done on the host while
unsharding (alongside the column un-permutation the host already does).

Device pipeline, per segment-ALIGNED row chunk (<=128 rows, cut at row
segment starts so no segment straddles chunks; ~5 chunks for H=512):

  matmul   psum[s, (wslot, c)] = sum_h F_k[h, s] * x[h, (wslot, l, c)]
           F_k[h, s] = 1/L_h(s) iff row h is in row-segment s of chunk k.
           The contraction over h does the row pooling; PSUM accumulation
           over the within-col-segment offset l (x is stored l-major per
           column-length class, host permutation) does the col pooling.
           Each input element passes the PE exactly once per pass; one
           full-width pass per chunk.
  evac     y_sb[s, (wslot, c)] = psum * (1/L_w) -- per column-class
           pieces, split between Vector and Scalar engines.
  store    y[s0_k:s1_k, :] <- y_sb, on the Activation HWDGE ring
           (x loads keep the SP ring).

The host un-permutes and broadcasts: out[h, w, c] =
y[rowseg(h), colslot(w)*CS + c], upcast fp16 -> fp32.  fp16 keeps HBM
traffic at 16.8 MB in + 4.2 MB out per core; the 2e-2 harness tolerance
leaves ~40x margin over fp16 rounding noise (col sums accumulate in
fp32 PSUM).
"""

import math
import numpy as np

H, W, C = 512, 512, 256
NCORES = 8
CS = C // NCORES  # 32 channels per core
P = 128
FW = W * CS       # row free size in elements (16384)
PSW = 512         # psum bank width in fp32 elems
CHUNK2 = 2048     # psum tile width (fp32 elems, four banks)
LOAD_SPLIT = 2    # DMAs per row-run of x (pipelining granularity)


def _segments(mask):
    m = np.asarray(mask).ravel()
    change = np.nonzero(m[1:] != m[:-1])[0] + 1
    bounds = np.concatenate([[0], change, [len(m)]]).astype(np.int64)
    return [(int(bounds[i]), int(bounds[i + 1])) for i in range(len(bounds) - 1)]


def _plan(row_segs, col_segs):
    """Host-side geometry planning shared by program build + data prep."""
    from collections import defaultdict

    S_h, S_w = len(row_segs), len(col_segs)

    # ---- column side: class-sorted device order -------------------------
    by_len = defaultdict(list)
    for t, (u, v) in enumerate(col_segs):
        by_len[v - u].append(t)

    wperm = np.empty(W, dtype=np.int64)   # dev w unit -> orig w
    perm_cols = []                        # slot -> orig col segment id
    off = 0
    slot_bases = []  # (L, n, slot0) per class, slot-ordered
    sl = 0
    for L in sorted(by_len):
        ts = by_len[L]
        for t in ts:
            u, v = col_segs[t]
            wperm[off:off + L] = np.arange(u, v)
            off += L
            perm_cols.append(t)
        slot_bases.append((L, len(ts), sl))
        sl += len(ts)
    assert off == W and sl == S_w

    # x element layout: class blocks in slot order, each [L, n, CS]
    # l-major; cls_x0[class] = elem offset of the block
    cls_x0 = []
    x0 = 0
    for (L, n, slot0) in slot_bases:
        cls_x0.append(x0)
        x0 += L * n * CS
    assert x0 == FW

    CPW = S_w * CS  # pooled row free size (fp32 psum elems)

    # ---- matmul piece groups, keyed by psum half (512 fp32 = 16 slots) --
    # group: (ps_off, width, [x_off per l]) -- one PSUM-accumulation set
    mm_groups = defaultdict(list)
    for ci_, (L, n, slot0) in enumerate(slot_bases):
        i = 0
        while i < n:
            hi = (slot0 + i) * CS // PSW
            room = ((hi + 1) * PSW - (slot0 + i) * CS) // CS
            take = min(room, n - i)
            x_offs = [
                cls_x0[ci_] + l * n * CS + i * CS for l in range(L)
            ]
            mm_groups[hi].append(
                ((slot0 + i) * CS, take * CS, x_offs)
            )
            i += take

    # ---- evacuation pieces, keyed by psum chunk (CHUNK2) ----------------
    slots_per_chunk = CHUNK2 // CS
    exp_pieces = defaultdict(list)
    for (L, n, slot0) in slot_bases:
        i = 0
        while i < n:
            ch = (slot0 + i) // slots_per_chunk
            room = (ch + 1) * slots_per_chunk - (slot0 + i)
            take = min(room, n - i)
            exp_pieces[ch].append((L, take, slot0 + i))
            i += take

    n_chunks = math.ceil(CPW / CHUNK2)

    # ---- row side: pack whole segments into bins of <=128 rows ----------
    # Each bin is one PE pass (one full-width stream of x), so fewer bins
    # directly cut PE time.  ceil(H/128) = 4 bins requires a perfect
    # packing; contiguous cuts rarely give one, so adopt the leftover
    # tail segments into bins with spare rows (extra row-runs only cost
    # an extra DMA slice each).
    seg_of_h = np.empty(H, dtype=np.int64)
    for s, (va, vb) in enumerate(row_segs):
        seg_of_h[va:vb] = s
    bins = _pack_bins(row_segs, seg_of_h)

    return dict(
        S_h=S_h, S_w=S_w, CPW=CPW,
        wperm=wperm, perm_cols=perm_cols, slot_bases=slot_bases,
        cls_x0=cls_x0,
        mm_groups=mm_groups, exp_pieces=exp_pieces, n_chunks=n_chunks,
        seg_of_h=seg_of_h, bins=bins,
    )


def _pack_bins(row_segs, seg_of_h):
    """Pack whole row segments into ceil(H/128) bins of <=128 rows.

    Returns a list of bins; each bin is a list of segment ids (ordered so
    that row-contiguous segments stay adjacent).  Falls back to greedy
    contiguous chunks (one extra bin) if no 4-bin packing is found.
    """
    S = len(row_segs)
    sizes = [b - a for a, b in row_segs]
    starts = [a for a, b in row_segs]
    n_bins = math.ceil(H / P)

    def greedy(cap):
        cuts = [0]
        while cuts[-1] < H:
            cands = [s for s in starts + [H] if cuts[-1] < s <= cuts[-1] + cap]
            if not cands:
                return None
            cuts.append(max(cands))
        return cuts

    sol = None
    # DFS over contiguous cut choices for the first n_bins bins; leftover
    # tail segments are first-fit-decreasing'd into bins' spare rows.
    cand_lim = 6

    def dfs(cuts):
        nonlocal sol
        if sol is not None:
            return
        if len(cuts) == n_bins + 1:
            c_end = cuts[-1]
            spare = [P - (cuts[i + 1] - cuts[i]) for i in range(n_bins)]
            tail = [s for s in range(S) if row_segs[s][0] >= c_end]
            assign = [[] for _ in range(n_bins)]
            for s in sorted(tail, key=lambda s: -sizes[s]):
                for b in range(n_bins):
                    if spare[b] >= sizes[s]:
                        spare[b] -= sizes[s]
                        assign[b].append(s)
                        break
                else:
                    return
            bins = []
            for b in range(n_bins):
                segs = [s for s in range(S)
                        if cuts[b] <= row_segs[s][0] < cuts[b + 1]]
                bins.append(segs + sorted(assign[b]))
            sol = bins
            return
        last = cuts[-1]
        cands = sorted(
            [s for s in starts if last < s <= last + P], reverse=True
        )[:cand_lim]
        for c in cands:
            dfs(cuts + [c])
            if sol is not None:
                return

    dfs([0])
    if sol is not None:
        return sol
    # fallback: greedy contiguous bins (usually n_bins + 1 of them),
    # smallest first so the tiny pass overlaps the first big load
    cuts = greedy(P)
    bins = []
    for i in range(len(cuts) - 1):
        bins.append([s for s in range(S)
                     if cuts[i] <= row_segs[s][0] < cuts[i + 1]])
    bins.sort(key=lambda segs: sum(sizes[s] for s in segs))
    return bins


def _bin_geometry(row_segs, bins):
    """Per-bin packing info: row runs, F matrix row/col maps.

    Returns list of dicts with:
      runs:  [(r0, r1, p0)] row ranges and their partition base
      rows:  total packed rows
      segs:  segment ids in bin order (psum slot / output row order)
    """
    out = []
    for segs in bins:
        runs = []
        p = 0
        cur = None
        for s in segs:
            a, b = row_segs[s]
            if cur is not None and a == cur[1]:
                cur = (cur[0], b, cur[2])
            else:
                if cur is not None:
                    runs.append(cur)
                    p += cur[1] - cur[0]
                cur = (a, b, p)
        if cur is not None:
            runs.append(cur)
            p += cur[1] - cur[0]
        assert p <= P
        out.append(dict(runs=runs, rows=p, segs=list(segs)))
    return out


def _build_program(row_segs, col_segs, plan):
    import concourse.mybir as mybir
    import concourse.tile as tile
    from concourse import bacc

    fp16 = mybir.dt.float16
    fp32 = mybir.dt.float32
    COPY = mybir.ActivationFunctionType.Copy

    S_h = plan["S_h"]
    CPW = plan["CPW"]
    geos = _bin_geometry(row_segs, plan["bins"])
    Kc = len(geos)
    n_chunks = plan["n_chunks"]
    n_halves = math.ceil(CPW / PSW)
    # pipelined stores: one DMA per group of psum chunks, issued as soon
    # as that group's evacuations are done (overlapping descriptors =
    # more DMA-engine parallelism; short tail after the last pass)
    STORE_SPLIT = 2
    store_bounds = sorted({
        min(CPW, round(i * n_chunks / STORE_SPLIT) * CHUNK2)
        for i in range(STORE_SPLIT + 1)
    })

    nc = bacc.Bacc()
    x = nc.dram_tensor("x", [H, FW], fp16, kind="ExternalInput")
    fM = nc.dram_tensor("fM", [Kc * P, P], fp16, kind="ExternalInput")
    y = nc.dram_tensor("y", [S_h, CPW], fp16, kind="ExternalOutput")

    with tile.TileContext(nc) as tc:
        with (
            tc.tile_pool(name="consts", bufs=1) as consts,
            tc.tile_pool(name="ys", bufs=3) as ypool,
            tc.tile_pool(name="ps2", bufs=2, space="PSUM") as ps2pool,
        ):
            # per-bin pooling matrices (stationary), on the SP ring
            fM_sb = []
            for k in range(Kc):
                t = consts.tile([P, P], fp16, name=f"fM{k}")
                nc.sync.dma_start(t[:], fM[k * P:(k + 1) * P, :])
                fM_sb.append(t)

            # x resident in SBUF, one tile per bin, loaded run-by-run in
            # column slices on the Activation HWDGE ring (stores keep the
            # SP ring)
            x_sb = []
            for k, g in enumerate(geos):
                t = consts.tile([P, FW], fp16, name=f"x{k}")
                step = FW // LOAD_SPLIT
                for s0_ in range(0, FW, step):
                    for (r0, r1, p0) in g["runs"]:
                        nc.scalar.dma_start(
                            t[p0:p0 + r1 - r0, s0_:s0_ + step],
                            x[r0:r1, s0_:s0_ + step],
                        )
                x_sb.append(t)

            # PE pre-touch of every stationary tile: later matmuls then
            # reach the operand without a DMA wait (keeps the LDWEIGHTS
            # sync-wait count within the ISA limit).  The warm tile is an
            # allocation of the ps2 ring (its banks are free again by the
            # time the second real psum tile needs them).
            ps_warm = ps2pool.tile([P, CHUNK2], fp32, tag="ps2",
                                   name="ps_warm")
            for t in fM_sb:
                nc.tensor.matmul(
                    ps_warm[:1, :1], t[:, :1], t[:, :1],
                    start=True, stop=True,
                )

            # --------- pooled pass per bin -------------------------------
            exp_rr = 0
            N0 = 0
            for k, g in enumerate(geos):
                rows = g["rows"]
                n_k = len(g["segs"])
                ot = ypool.tile([P, CPW], fp16, tag="ot", name=f"ot{k}")
                si = 0
                for ci in range(n_chunks):
                    c0 = ci * CHUNK2
                    ps2 = ps2pool.tile([P, CHUNK2], fp32, tag="ps2",
                                       name=f"ps2_{k}_{ci}")
                    hpc = CHUNK2 // PSW
                    for hi in range(hpc * ci, min(hpc * (ci + 1), n_halves)):
                        for (ps_off, width, x_offs) in plan["mm_groups"][hi]:
                            tot = len(x_offs)
                            for idx, x_off in enumerate(x_offs):
                                nc.tensor.matmul(
                                    ps2[:n_k,
                                        ps_off - c0:
                                        ps_off - c0 + width],
                                    fM_sb[k][:rows, :n_k],
                                    x_sb[k][:rows, x_off:x_off + width],
                                    start=(idx == 0),
                                    stop=(idx == tot - 1),
                                )
                    for (L, n, slot0) in plan["exp_pieces"][ci]:
                        src = ps2[:n_k, slot0 * CS - c0:
                                  (slot0 + n) * CS - c0]
                        dst = ot[:n_k, slot0 * CS:(slot0 + n) * CS]
                        if exp_rr % 2 == 0:
                            nc.vector.tensor_scalar_mul(dst, src, 1.0 / L)
                        else:
                            nc.scalar.activation(dst, src, COPY,
                                                 scale=1.0 / L)
                        exp_rr += 1
                    while (si < STORE_SPLIT
                           and (ci + 1) * CHUNK2 >= store_bounds[si + 1]):
                        w0, w1 = store_bounds[si], store_bounds[si + 1]
                        nc.sync.dma_start(y[N0:N0 + n_k, w0:w1],
                                          ot[:n_k, w0:w1])
                        si += 1
                N0 += n_k

    nc.compile()
    nc.finalize()
    return nc


def _prep_host(input, h_mask, v_mask):
    """Returns (nc, in_maps, plan) ready for execution."""
    row_segs = _segments(h_mask)
    col_segs = _segments(v_mask)
    plan = _plan(row_segs, col_segs)
    nc = _build_program(row_segs, col_segs, plan)
    in_maps = _make_in_maps(input, row_segs, plan)
    return nc, in_maps, plan


def _make_in_maps(input, row_segs, plan):
    # per-bin pooling matrices: fM[k*128 + packed_row, local_slot] = 1/len
    geos = _bin_geometry(row_segs, plan["bins"])
    Kc = len(geos)
    fM = np.zeros((Kc * P, P), dtype=np.float16)
    for k, g in enumerate(geos):
        slot = {s: i for i, s in enumerate(g["segs"])}
        for (r0, r1, p0) in g["runs"]:
            for h in range(r0, r1):
                s = int(plan["seg_of_h"][h])
                va, vb = row_segs[s]
                fM[k * P + p0 + (h - r0), slot[s]] = np.float16(
                    1.0 / (vb - va))

    # device x layout: class blocks in slot order, each [L, n, C] l-major
    xp16 = np.asarray(input)[0].astype(np.float16)  # [H, W, C]
    parts = []
    pos = 0
    for (L, n, slot0) in plan["slot_bases"]:
        cols = plan["wperm"][pos:pos + n * L]
        pos += n * L
        blk = xp16[:, cols, :].reshape(H, n, L, C)
        parts.append(np.ascontiguousarray(blk.transpose(0, 2, 1, 3)))

    in_maps = []
    for k in range(NCORES):
        xc = np.concatenate(
            [p[:, :, :, k * CS:(k + 1) * CS].reshape(H, -1) for p in parts],
            axis=1,
        )
        in_maps.append({"x": np.ascontiguousarray(xc), "fM": fM})
    return in_maps


# stash for test.py introspection
LAST_RESULT = {}
_EXEC_CACHE = {}


def _make_executable(nc):
    """Build a reusable sharded jit callable for this program.

    Mirrors bass2jax.run_bass_via_pjrt's multi-core branch but keeps the
    jitted function so repeated calls skip retrace/recompile (and so the
    test harness can time steady-state executions).
    """
    import jax
    import concourse.mybir as mybir
    from concourse import bass2jax
    from jax.sharding import Mesh, PartitionSpec
    from jax.experimental.shard_map import shard_map

    bass2jax.install_neuronx_cc_hook()

    partition_name = (
        nc.partition_id_tensor.name if nc.partition_id_tensor else None
    )
    in_names, out_names, out_shapes, out_dtypes = [], [], [], []
    for alloc in nc.m.functions[0].allocations:
        if not isinstance(alloc, mybir.MemoryLocationSet):
            continue
        name = alloc.memorylocations[0].name
        if alloc.kind == "ExternalInput":
            if name != partition_name:
                in_names.append(name)
        elif alloc.kind == "ExternalOutput":
            out_names.append(name)
            out_shapes.append(tuple(alloc.tensor_shape))
            out_dtypes.append(mybir.dt.np(alloc.dtype))
    out_avals = tuple(
        jax.core.ShapedArray(s, d) for s, d in zip(out_shapes, out_dtypes)
    )
    n_params = len(in_names)
    n_outs = len(out_names)
    all_names = in_names + out_names
    if partition_name is not None:
        all_names = all_names + [partition_name]

    def _body(*args):
        operands = list(args)
        if partition_name is not None:
            operands.append(bass2jax.partition_id_tensor())
        outs = bass2jax._bass_exec_p.bind(
            *operands,
            out_avals=out_avals,
            in_names=tuple(all_names),
            out_names=tuple(out_names),
            lowering_input_output_aliases=(),
            sim_require_finite=True,
            sim_require_nnan=True,
            nc=nc,
        )
        return tuple(outs)

    devices = jax.devices()[:NCORES]
    mesh = Mesh(np.asarray(devices), ("core",))
    donate = tuple(range(n_params, n_params + n_outs))
    sharded = jax.jit(
        shard_map(
            _body,
            mesh=mesh,
            in_specs=(PartitionSpec("core"),) * (n_params + n_outs),
            out_specs=(PartitionSpec("core"),) * n_outs,
            check_rep=False,
        ),
        donate_argnums=donate,
        keep_unused=True,
    )

    def run(in_maps):
        concat_in = [
            np.concatenate([m[name] for m in in_maps], axis=0)
            for name in in_names
        ]
        concat_zeros = [
            np.zeros((NCORES * s[0], *s[1:]), d)
            for s, d in zip(out_shapes, out_dtypes)
        ]
        out_arrs = sharded(*concat_in, *concat_zeros)
        return [
            {
                name: np.asarray(out_arrs[i]).reshape(
                    NCORES, *out_shapes[i]
                )[c]
                for i, name in enumerate(out_names)
            }
            for c in range(NCORES)
        ]

    return run


def _get_run(input, h_mask, v_mask):
    key = (np.asarray(h_mask).tobytes(), np.asarray(v_mask).tobytes())
    if key not in _EXEC_CACHE:
        nc, in_maps, plan = _prep_host(
            np.asarray(input), np.asarray(h_mask), np.asarray(v_mask)
        )
        LAST_RESULT["nc"] = nc
        _EXEC_CACHE[key] = (_make_executable(nc), plan)
    else:
        row_segs = _segments(h_mask)
        plan = _EXEC_CACHE[key][1]
        in_maps = _make_in_maps(np.asarray(input), row_segs, plan)
    return _EXEC_CACHE[key][0], in_maps


def kernel(input, h_mask, v_mask):
    run, in_maps = _get_run(input, h_mask, v_mask)
    results = run(in_maps)
    LAST_RESULT["results"] = results

    key = (np.asarray(h_mask).tobytes(), np.asarray(v_mask).tobytes())
    plan = _EXEC_CACHE[key][1]
    S_h, S_w = plan["S_h"], plan["S_w"]

    # broadcast-back gather: out[h, w, c] = y[devrow(rowseg(h)), colslot(w), c]
    slot_of_seg = np.empty(S_w, dtype=np.int64)
    slot_of_seg[np.asarray(plan["perm_cols"], dtype=np.int64)] = \
        np.arange(S_w)
    col_segs = _segments(v_mask)
    seg_of_w = np.empty(W, dtype=np.int64)
    for t, (u, v) in enumerate(col_segs):
        seg_of_w[u:v] = t
    col_ix = slot_of_seg[seg_of_w]          # [W] -> slot
    devrow_of_seg = np.empty(S_h, dtype=np.int64)
    n = 0
    for segs in plan["bins"]:
        for s in segs:
            devrow_of_seg[s] = n
            n += 1
    assert n == S_h
    row_ix = devrow_of_seg[plan["seg_of_h"]]  # [H] -> device y row

    out = np.empty((H, W, C), dtype=np.float32)
    for k in range(NCORES):
        yk = results[k]["y"].astype(np.float32).reshape(S_h, S_w, CS)
        out[:, :, k * CS:(k + 1) * CS] = yk[row_ix][:, col_ix]
    return out[None]
